# revision 51
# baseline (speedup 1.0000x reference)
"""Trainium2 Bass kernel for nn_Encoder (bidirectional-LSTM encoder + attention).

Strategy: data-parallel over batch B=128 across 8 cores (16 batch elems/core).
Each core runs the full pipeline locally (embedding gather, both LSTM
directions for sentence+target, attention, output head). No cross-core
communication; host concatenates the per-core [16, 3] outputs.

v3: the input projections are folded into the recurrence itself (3 extra
matmuls per step accumulate Wih@x + bias into the gate PSUM before the Whh
matmuls), the embedding table is gathered in bf16 and transposed by the DMA
xbar instead of the PE, and gathers are chunked time-major so all four LSTM
chains (sen fwd/bwd, tgt fwd/bwd) start as soon as their first/last time
chunks land. All LSTM state is gate-transposed ([gate_dim, batch]); tanh is
computed via sigmoid (tanh(x) = 2*sigmoid(2x) - 1, with g-gate weight rows
pre-scaled by 2 on the host) so the ACT table never switches mid-recurrence.
"""

import sys

sys.path.insert(0, "/opt/trn_rl_repo")

import numpy as np
import ml_dtypes

import concourse.bass as bass
import concourse.mybir as mybir
import concourse.tile as tile
from concourse.bass_utils import run_bass_kernel_spmd
from concourse.masks import make_identity
from concourse.vector_clock import ScopedClock

V, E, H, OUT = 100000, 300, 256, 3
B, LS, LT = 128, 128, 8
NCORES = 8
BL = B // NCORES  # 16 batch elements per core
G4 = 4 * H  # 1024 (gate dim)
NTOK_S = BL * LS  # 2048 sentence tokens per core
NTOK_T = BL * LT  # 128 target tokens per core
NCH_S = NTOK_S // 128  # 16 time-major chunks, 8 timesteps each
TS_CH = LS // NCH_S  # 8 timesteps per sentence chunk
EP = 384  # embedding row padded to 3x128; col 300 = 1.0 (bias row), rest 0

dt = mybir.dt
AF = mybir.ActivationFunctionType
ALU = mybir.AluOpType
f32 = dt.float32
bf16 = dt.bfloat16
f8 = dt.float8e4


# ---------------------------------------------------------------------------
# Workaround: this walrus build rejects >2 semaphore waits on one CTRL
# instruction; split the TileContext exit-drain waits onto individual nops.
def _patched_drain_and_barrier(self, tick_clock, wait_clock):
    nc = self.nc
    collect = nc.sync.nop()
    wait_clock.add_sem_waits(collect.ins, ScopedClock({None: tick_clock.global_clock}))
    si = collect.ins.sync_info
    waits = list(si.on_wait) if si and si.on_wait else []
    if len(waits) > 1:
        si.on_wait = waits[:1]
        for w in waits[1:]:
            nop = nc.sync.nop()
            if nop.ins.sync_info is None:
                nop.ins.sync_info = mybir.SyncInfo(on_wait=[w], on_update=[])
            else:
                nop.ins.sync_info.on_wait = [w]
    nc.sync.drain()
    nc.all_engine_barrier()
    popped = nc._tile_sem_poison_stack.pop()
    assert popped is self._sem_poison
    nc.clear_and_free_semaphores(list(self.sems.allocated().values()))
    nc.all_engine_barrier()


tile.TileContext._drain_and_barrier = _patched_drain_and_barrier


def _split_sync_waits(nc, max_waits=1):
    """Hoist excess semaphore waits (>max_waits per instruction) onto
    same-engine NoOp instructions inserted just before, preserving engine
    stream order (this walrus build encodes at most 2 waits/instruction)."""
    import bass_rust as _br

    ctr = [0]
    for fn in nc.m.functions:
        for bb in fn.blocks:
            out = []
            changed = False
            for inst in bb.instructions:
                si = getattr(inst, "sync_info", None)
                if si is not None and si.on_wait and len(si.on_wait) > max_waits:
                    waits = list(si.on_wait)
                    si.on_wait = waits[:max_waits]
                    rest = waits[max_waits:]
                    for j in range(0, len(rest), max_waits):
                        ctr[0] += 1
                        nop = _br.InstNoOp(name=f"WS-{ctr[0]}", ins=[], outs=[])
                        nop.engine = inst.engine
                        nop.sync_info = mybir.SyncInfo(
                            on_wait=rest[j : j + max_waits], on_update=[]
                        )
                        out.append(nop)
                    changed = True
                out.append(inst)
            if changed:
                bb.instructions = out


# ---------------------------------------------------------------------------


def _emit_lstm(nc, pools, T, d, xT_v, wih, whh, bias_sb, expand, hsT_v, tag):
    """Emit one LSTM direction's recurrence. Gates for step t are built in
    PSUM: a full-region bias-inject matmul (start=True — one accumulation
    group per step per the 2KB zero-region rule), then sum_k Wih[k] @ x[t]
    plus the two recurrent Whh tiles applied to h[t-1].

    T: sequence length. d: 0=forward, 1=backward (within this LSTM's hsT).
    xT_v:   [128, 3, T, BL] bf16 view of transposed embeddings (time-major).
    wih:    [128, 3, G4] bf16 input weights (lhsT tiles).
    whh:    [128, 2, G4] fp8 recurrent weights (lhsT tiles), host-scaled x2
            (hidden states are stored halved).
    bias_sb: [8, 128] bf16, row m = bias[m*128 : (m+1)*128].
    expand: [8, 128] bf16 constant, expand[k, m*16 + b] = (k == m).
    hsT_v:  [128, 4, BL, T] bf16 view; this direction writes slots 2d, 2d+1
            holding h/2 (downstream consumers compensate).
    """
    spool, pgates = pools
    CH = 4  # psum chunk: 4 timesteps per bank
    ctag = f"c_{tag}"
    steps = range(T - 1, -1, -1) if d == 1 else range(T)
    c_prev = None
    for si, t in enumerate(steps):
        tl = si % CH
        if tl == 0:
            g = pgates.tile([128, CH * 128], f32, tag=f"g_{tag}", name=f"g_{tag}_{si}")
        gv = g[:, tl * 128 : (tl + 1) * 128]
        gv3 = gv.rearrange("p (m b) -> p m b", b=BL)
        # bias inject over the whole step region opens the accumulation group
        nc.tensor.matmul(gv3, bias_sb, expand.rearrange("p (m b) -> p m b", b=BL),
                         start=True, stop=False)
        # input projection directly into PSUM (off the critical path),
        # then the recurrent matmuls accumulate on top (h[-1] = 0: skipped)
        for m in range(8):
            for k in range(3):
                kn = 128 if k < 2 else 44
                nc.tensor.matmul(
                    gv3[:, m, :],
                    wih[0:kn, k, m * 128 : (m + 1) * 128],
                    xT_v[0:kn, k, t, :],
                    start=False,
                    stop=(si == 0 and m == 7 and k == 2),
                )
        if si > 0:
            for m in range(8):
                nc.tensor.matmul(
                    gv3[:, m, :],
                    whh[:, :, m * 128 : (m + 1) * 128],
                    h8_prev,
                    start=False,
                    stop=(m == 7),
                    perf_mode=mybir.MatmulPerfMode.DoubleRow,
                )
        # all four gates through sigmoid in one op (g-gate pre-scaled by 2)
        sig = spool.tile([128, 128], bf16, tag=f"sig_{tag}", name=f"sig_{tag}_{si}")
        nc.scalar.activation(sig, gv, AF.Sigmoid)
        # c = sig(f)*c + sig(i)*tanh(g);  tanh(g) = 2*sig(2g)-1, so
        # mh := (sig(2g)-0.5)*sig(i) = sig(i)*tanh(g)/2
        cn = spool.tile([128, 32], bf16, tag=ctag, name=f"c_{tag}_{si}")
        mh = spool.tile([128, 32], bf16, tag=f"mh_{tag}", name=f"mh_{tag}_{si}")
        nc.vector.scalar_tensor_tensor(
            mh, sig[:, 64:96], -0.5, sig[:, 0:32], op0=ALU.add, op1=ALU.mult
        )
        if si == 0:
            nc.vector.tensor_scalar(cn, mh, 2.0, None, op0=ALU.mult)
        else:
            t2 = spool.tile([128, 32], bf16, tag=f"t2_{tag}", name=f"t2_{tag}_{si}")
            nc.vector.tensor_tensor(t2, sig[:, 32:64], c_prev, op=ALU.mult)
            nc.vector.scalar_tensor_tensor(cn, mh, 2.0, t2, op0=ALU.mult, op1=ALU.add)
        c_prev = cn
        # stored h/2 = (sig(2c)-0.5) * sig(o)  [= sig(o)*tanh(c)/2]
        # fp8 ring tile feeds the next step's DoubleRow matmuls; the bf16
        # hsT copy for attention/phase-3 happens off the critical chain
        vt = spool.tile([128, 32], bf16, tag=f"v_{tag}", name=f"v_{tag}_{si}")
        nc.scalar.activation(vt, cn, AF.Sigmoid, scale=2.0)
        h8 = spool.tile([128, 2, BL], f8, tag=f"h8_{tag}", name=f"h8_{tag}_{si}")
        nc.vector.scalar_tensor_tensor(
            h8,
            vt.rearrange("p (s b) -> p s b", b=BL), -0.5,
            sig[:, 96:128].rearrange("p (s b) -> p s b", b=BL),
            op0=ALU.add, op1=ALU.mult,
        )
        nc.vector.tensor_copy(hsT_v[:, 2 * d : 2 * d + 2, :, t], h8)
        h8_prev = h8
        yield


def _interleave(*gens, lead=0):
    """Drive per-step generators round-robin so the emitted instruction
    streams alternate between chains step by step (keeps the engine queues
    phase-balanced instead of relying on the scheduler's tie-breaks). The
    first generator is advanced `lead` extra steps up front to bias the two
    chains to a half-period phase offset."""
    alive = list(gens)
    for _ in range(lead):
        next(alive[0])
    while alive:
        nxt = []
        for gen in alive:
            try:
                next(gen)
                nxt.append(gen)
            except StopIteration:
                pass
        alive = nxt


def _build_program():
    nc = bass.Bass("TRN2", target_bir_lowering=False, debug=False)

    # --- DRAM I/O -----------------------------------------------------------
    d_emb = nc.dram_tensor("emb", [V, EP], bf16, kind="ExternalInput").ap()
    # col 0 = target chunk, cols 1..16 = sentence time chunks
    d_idx = nc.dram_tensor("idx", [128, 1 + NCH_S], dt.int32, kind="ExternalInput").ap()
    d_wih = {}
    d_whh = {}
    for nm in ("sf", "sb", "tf", "tb"):
        d_wih[nm] = nc.dram_tensor(f"wih_{nm}", [3, 128, G4], f8, kind="ExternalInput").ap()
        d_whh[nm] = nc.dram_tensor(f"whh_{nm}", [2, 128, G4], f8, kind="ExternalInput").ap()
    # cols 0:512 = biases (sf, sb, tf, tb), cols 512:640 = m-expansion matrix
    d_bx = nc.dram_tensor("bx", [8, 640], bf16, kind="ExternalInput").ap()
    d_wout = nc.dram_tensor("woutT", [4, 128, OUT], bf16, kind="ExternalInput").ap()
    d_bout = nc.dram_tensor("boutT", [OUT, 1], f32, kind="ExternalInput").ap()
    d_out = nc.dram_tensor("out", [BL, OUT], f32, kind="ExternalOutput").ap()

    with tile.TileContext(nc) as tc:
        with (
            tc.tile_pool(name="cpool", bufs=1) as cpool,
            tc.tile_pool(name="spool", bufs=3) as spool,
            tc.tile_pool(name="gpool", bufs=6) as gpool,
            tc.tile_pool(name="pgates", bufs=2, space="PSUM") as pgates,
        ):
            # --- constants / weights into SBUF ------------------------------
            # index + bias loads first on SP (gathers depend on idx); big
            # weight loads split across the ACT and DVE DMA queues so the
            # first recurrence steps aren't starved behind a serial SP queue.
            idx = cpool.tile([128, 1 + NCH_S], dt.int32, name="idx")
            nc.sync.dma_start(idx, d_idx)
            bx = cpool.tile([8, 640], bf16, name="bx")
            nc.sync.dma_start(bx, d_bx)
            wih = {}
            whh = {}
            bias_sb = {}
            for nm, eng in (("tf", nc.sync), ("tb", nc.scalar), ("sf", nc.sync), ("sb", nc.scalar)):
                wt = cpool.tile([128, 3, G4], f8, name=f"wih_{nm}_sb")
                eng.dma_start(wt, d_wih[nm].rearrange("k p m -> p k m"))
                wih[nm] = wt
                ht = cpool.tile([128, 2, G4], f8, name=f"whh_{nm}_sb")
                eng.dma_start(ht, d_whh[nm].rearrange("k p m -> p k m"))
                whh[nm] = ht
            for i, nm in enumerate(("sf", "sb", "tf", "tb")):
                bias_sb[nm] = bx[:, 128 * i : 128 * (i + 1)]
            expand = bx[:, 512:640]
            # preload the sigmoid ACT table after the ACT-queue weight DMAs
            # (ready before the first real sig, without delaying the loads)
            scr = cpool.tile([1, 1], f32, name="scr")
            nc.vector.memset(scr, 0.0)
            nc.scalar.activation(scr, scr, AF.Sigmoid)
            wout_sb = cpool.tile([128, 4, OUT], bf16, name="wout_sb")
            nc.sync.dma_start(wout_sb, d_wout.rearrange("k p m -> p k m"))
            bout_sb = cpool.tile([OUT, 1], f32, name="bout_sb")
            nc.sync.dma_start(bout_sb, d_bout)

            ident = cpool.tile([128, 128], f32, name="ident")
            make_identity(nc, ident)
            ones = cpool.tile([128, 128], f32, name="ones")
            nc.gpsimd.memset(ones, 1.0)

            # --- persistent activations -------------------------------------
            xT_s = cpool.tile([128, 3, NTOK_S], bf16, name="xT_s")
            xT_t = cpool.tile([128, 3, NTOK_T], bf16, name="xT_t")
            hsT = cpool.tile([128, 4, NTOK_S], bf16, name="hsT")
            ttT = cpool.tile([128, 4, NTOK_T], bf16, name="ttT")

            # --- phase 1: gather + DMA transpose (time-major chunks) --------
            # xT free layout is time-major (col = t*BL + b), so chunk j of
            # the sentence (timesteps [8j, 8j+8) for all 16 batch elems) is a
            # contiguous 128-col block and every DMA-transpose output is a
            # plain 2D [128, 128] tile. fwd starts after chunk 0, bwd after
            # chunk 15, so the gathers are ordered from both ends inward.
            def gather_chunk(col, j, which):
                gx = gpool.tile([128, EP], bf16, tag="gx", name=f"gx_{which}_{j}")
                nc.gpsimd.indirect_dma_start(
                    out=gx,
                    out_offset=None,
                    in_=d_emb[:, :],
                    in_offset=bass.IndirectOffsetOnAxis(ap=idx[:, col : col + 1], axis=0),
                )
                for k in range(3):
                    if which == "s":
                        out = xT_s[:, k, j * 128 : (j + 1) * 128]
                    else:
                        out = xT_t[:, k, :]
                    nc.sync.dma_start_transpose(out, gx[:, k * 128 : (k + 1) * 128])

            order = []
            lo, hi = 0, NCH_S - 1
            while lo <= hi:
                order.append(lo)
                if hi != lo:
                    order.append(hi)
                lo += 1
                hi -= 1
            gather_chunk(0, 0, "t")
            for j in order:
                gather_chunk(1 + j, j, "s")

            # --- phase 2: recurrences ---------------------------------------
            lpools = (spool, pgates)
            ttT_v = ttT.rearrange("p s (b t) -> p s b t", b=BL)
            xTt_v = xT_t.rearrange("p k (t b) -> p k t b", b=BL)
            xTs_v = xT_s.rearrange("p k (t b) -> p k t b", b=BL)
            hsT_v = hsT.rearrange("p s (b t) -> p s b t", b=BL)
            _interleave(
                _emit_lstm(nc, lpools, LT, 0, xTt_v, wih["tf"], whh["tf"], bias_sb["tf"], expand, ttT_v, "tf"),
                _emit_lstm(nc, lpools, LT, 1, xTt_v, wih["tb"], whh["tb"], bias_sb["tb"], expand, ttT_v, "tb"),
            )
            _interleave(
                _emit_lstm(nc, lpools, LS, 0, xTs_v, wih["sf"], whh["sf"], bias_sb["sf"], expand, hsT_v, "sf"),
                _emit_lstm(nc, lpools, LS, 1, xTs_v, wih["sb"], whh["sb"], bias_sb["sb"], expand, hsT_v, "sb"),
                lead=1,
            )

        # --- phase 3: attention + output head ------------------------------
        with (
            tc.tile_pool(name="apool", bufs=1) as apool,
            tc.tile_pool(name="patt", bufs=1, space="PSUM") as patt,
        ):
            # A[b,s,t] stored as [s(part), b*8+t]
            a3 = patt.tile([128, 128], f32, name="a3")
            for b in range(BL):
                for k in range(4):
                    nc.tensor.matmul(
                        a3[:, b * 8 : (b + 1) * 8],
                        hsT[:, k, b * 128 : (b + 1) * 128],
                        ttT[:, k, b * 8 : (b + 1) * 8],
                        start=(k == 0),
                        stop=(k == 3),
                    )
            expA = apool.tile([128, 128], f32, name="expA")
            nc.scalar.activation(expA, a3, AF.Exp, scale=4.0)
            expA_v = expA.rearrange("p (b t) -> p b t", t=LT)
            # row softmax (over t) then mean over s, divided by col sums (over s)
            rsum = apool.tile([128, BL], f32, name="rsum")
            nc.vector.tensor_reduce(rsum, expA_v, axis=mybir.AxisListType.X, op=ALU.add)
            rr = apool.tile([128, BL], f32, name="rr")
            nc.vector.reciprocal(rr, rsum)
            rnorm = apool.tile([128, 128], f32, name="rnorm")
            rr_b = bass.AP(tensor=rr.tensor, offset=rr.offset, ap=list(rr.ap) + [[0, LT]])
            nc.vector.tensor_tensor(rnorm.rearrange("p (b t) -> p b t", t=LT), expA_v, rr_b, op=ALU.mult)
            rvp = patt.tile([1, 128], f32, name="rvp")
            nc.tensor.matmul(rvp, ones[:, 0:1], rnorm, start=True, stop=True)
            csum = patt.tile([1, 128], f32, name="csum")
            nc.tensor.matmul(csum, ones[:, 0:1], expA, start=True, stop=True)
            rc = apool.tile([1, 128], f32, name="rc")
            nc.vector.reciprocal(rc, csum)
            q = apool.tile([1, 128], f32, name="q")
            nc.vector.scalar_tensor_tensor(q, rvp, 1.0 / LS, rc, op0=ALU.mult, op1=ALU.mult)
            qbc = patt.tile([128, 128], f32, name="qbc")
            nc.tensor.matmul(qbc, ones[0:1, :], q, start=True, stop=True)
            attw = apool.tile([128, 128], f32, name="attw")
            nc.vector.tensor_tensor(attw, expA, qbc, op=ALU.mult)
            attnT = apool.tile([128, BL], f32, name="attnT")
            nc.vector.tensor_reduce(attnT, attw.rearrange("p (b t) -> p b t", t=LT), axis=mybir.AxisListType.X, op=ALU.add)
            attnb = apool.tile([128, BL], bf16, name="attnb")
            nc.vector.tensor_copy(attnb, attnT)

            # de-transpose sen_h for the score contraction; issue cost is
            # ~112ns each, so split the 64 transposes across both DMA queues
            sen_h = apool.tile([128, BL, 4 * 128], bf16, name="sen_h")
            for b in range(BL):
                for k in range(4):
                    eng = nc.sync if (b * 4 + k) % 2 == 0 else nc.scalar
                    eng.dma_start_transpose(
                        sen_h[:, b, k * 128 : (k + 1) * 128], hsT[:, k, b * 128 : (b + 1) * 128]
                    )
            scoT = patt.tile([128, 4 * BL], f32, name="scoT")
            for b in range(BL):
                for mh in range(4):
                    nc.tensor.matmul(
                        scoT[:, b * 4 + mh : b * 4 + mh + 1],
                        sen_h[:, b, mh * 128 : (mh + 1) * 128],
                        attnb[:, b : b + 1],
                        start=True,
                        stop=True,
                    )
            scoB = apool.tile([128, 4 * BL], bf16, name="scoB")
            nc.scalar.activation(scoB, scoT, AF.Copy)
            lgT = patt.tile([OUT, BL], f32, name="lgT")
            for mh in range(4):
                nc.tensor.matmul(
                    lgT, wout_sb[:, mh, :], scoB[:, mh :: 4], start=(mh == 0), stop=(mh == 3)
                )
            lgsb = apool.tile([OUT, BL], f32, name="lgsb")
            nc.scalar.activation(lgsb, lgT, AF.Identity, bias=bout_sb[0:OUT, 0:1])
            lg2 = patt.tile([BL, OUT], f32, name="lg2")
            nc.tensor.transpose(lg2, lgsb, ident[0:OUT, 0:OUT])
            eo = apool.tile([BL, OUT], f32, name="eo")
            nc.scalar.activation(eo, lg2, AF.Exp)
            es = apool.tile([BL, 1], f32, name="es")
            nc.vector.tensor_reduce(es, eo, axis=mybir.AxisListType.X, op=ALU.add)
            er = apool.tile([BL, 1], f32, name="er")
            nc.vector.reciprocal(er, es)
            res = apool.tile([BL, OUT], f32, name="res")
            nc.vector.tensor_scalar(res, eo, er, None, op0=ALU.mult)
            nc.sync.dma_start(d_out, res)

    _split_sync_waits(nc)
    return nc


_CACHE = {}


def _get_program():
    if "nc" not in _CACHE:
        _CACHE["nc"] = _build_program()
    return _CACHE["nc"]


def prepare_in_maps(inputs):
    """Host-side prep: shard + repack inputs into per-core in_maps."""
    bf = ml_dtypes.bfloat16
    sen = np.asarray(inputs["sentence_source"]).astype(np.int32)  # [B, LS]
    tgt = np.asarray(inputs["target_source"]).astype(np.int32)  # [B, LT]
    emb = np.asarray(inputs["emb_W"], dtype=np.float32).copy()
    emb[0, :] = 0.0  # padding_idx
    emb_bf = np.zeros((V, EP), dtype=bf)
    emb_bf[:, :E] = emb.astype(bf)
    emb_bf[:, E] = 1.0  # multiplies the bias row of wih (k=2, row 44)

    def pack_wih(nm):
        W = np.asarray(inputs[f"Wih_{nm}"], dtype=np.float32).T.copy()  # [300, 1024]
        W[:, 2 * H : 3 * H] *= 2.0  # g-gate: tanh via sigmoid
        e4 = ml_dtypes.float8_e4m3
        pack = np.zeros((3, 128, G4), dtype=e4)
        pack[0] = W[0:128].astype(e4)
        pack[1] = W[128:256].astype(e4)
        pack[2, 0:44] = W[256:300].astype(e4)
        return pack

    def pack_bias(nm):
        bias = (
            np.asarray(inputs[f"bih_{nm}"], dtype=np.float32)
            + np.asarray(inputs[f"bhh_{nm}"], dtype=np.float32)
        ).copy()
        bias[2 * H : 3 * H] *= 2.0
        return np.ascontiguousarray(bias.reshape(8, 128).astype(bf))

    def pack_whh(nm):
        W = np.asarray(inputs[f"Whh_{nm}"], dtype=np.float32).T.copy()  # [256, 1024]
        W *= 2.0  # hidden states are stored halved
        W[:, 2 * H : 3 * H] *= 2.0
        return np.ascontiguousarray(W.reshape(2, 128, G4).astype(ml_dtypes.float8_e4m3))

    shared = {"emb": emb_bf}
    bx = np.zeros((8, 640), dtype=bf)
    for i, nm in enumerate(("sf", "sb", "tf", "tb")):
        shared[f"wih_{nm}"] = pack_wih(nm)
        shared[f"whh_{nm}"] = pack_whh(nm)
        bx[:, 128 * i : 128 * (i + 1)] = pack_bias(nm)
    bx[:, 512:640] = np.repeat(np.eye(8), BL, axis=1).astype(bf)
    shared["bx"] = np.ascontiguousarray(bx)
    Wout = np.asarray(inputs["Wout"], dtype=np.float32) * 2.0  # [3, 512]; sen_h halved
    shared["woutT"] = np.ascontiguousarray(Wout.T.reshape(4, 128, OUT).astype(bf))
    shared["boutT"] = np.asarray(inputs["bout"], dtype=np.float32).reshape(OUT, 1)

    in_maps = []
    for c in range(NCORES):
        sl = slice(c * BL, (c + 1) * BL)
        m = dict(shared)
        # sentence chunk j = timesteps [8j, 8j+8) across all 16 batch elems,
        # time-major within the chunk (matches the xT free layout t*16+b)
        sh = sen[sl]  # [16, 128]
        cols = [tgt[sl].T.reshape(-1)]
        cols += [sh[:, TS_CH * j : TS_CH * (j + 1)].T.reshape(-1) for j in range(NCH_S)]
        m["idx"] = np.ascontiguousarray(np.stack(cols, axis=1))  # [128, 17]
        in_maps.append(m)
    return in_maps


def kernel(**inputs) -> np.ndarray:
    nc = _get_program()
    in_maps = prepare_in_maps(inputs)
    r = run_bass_kernel_spmd(nc, in_maps, core_ids=list(range(NCORES)))
    return np.concatenate([r.results[c]["out"] for c in range(NCORES)], axis=0)


if __name__ == "__main__":
    print("building program...")
    nc = _get_program()
    print("build OK")


# revision 52
# speedup vs baseline: 2.4982x; 2.4982x over previous
"""Trainium2 Bass kernel for nn_Encoder (bidirectional-LSTM encoder + attention).

Strategy: data-parallel over batch B=128 across 8 cores (16 batch elems/core).
Each core runs the full pipeline locally (embedding gather, both LSTM
directions for sentence+target, attention, output head). No cross-core
communication; host concatenates the per-core [16, 3] outputs.

v3: the input projections are folded into the recurrence itself (3 extra
matmuls per step accumulate Wih@x + bias into the gate PSUM before the Whh
matmuls), the embedding table is gathered in bf16 and transposed by the DMA
xbar instead of the PE, and gathers are chunked time-major so all four LSTM
chains (sen fwd/bwd, tgt fwd/bwd) start as soon as their first/last time
chunks land. All LSTM state is gate-transposed ([gate_dim, batch]); tanh is
computed via sigmoid (tanh(x) = 2*sigmoid(2x) - 1, with g-gate weight rows
pre-scaled by 2 on the host) so the ACT table never switches mid-recurrence.
"""

import sys

sys.path.insert(0, "/opt/trn_rl_repo")

import numpy as np
import ml_dtypes

import concourse.bass as bass
import concourse.mybir as mybir
import concourse.tile as tile
from concourse.bass_utils import run_bass_kernel_spmd
from concourse.masks import make_identity
from concourse.vector_clock import ScopedClock

V, E, H, OUT = 100000, 300, 256, 3
B, LS, LT = 128, 128, 8
NCORES = 8
BL = B // NCORES  # 16 batch elements per core
G4 = 4 * H  # 1024 (gate dim)
NTOK_S = BL * LS  # 2048 sentence tokens per core
NTOK_T = BL * LT  # 128 target tokens per core
NCH_S = NTOK_S // 128  # 16 time-major chunks, 8 timesteps each
TS_CH = LS // NCH_S  # 8 timesteps per sentence chunk
EP = 320  # embedding row padded; k-slices 0:128, 128:256, 192:320 (overlap)

dt = mybir.dt
AF = mybir.ActivationFunctionType
ALU = mybir.AluOpType
f32 = dt.float32
bf16 = dt.bfloat16
f8 = dt.float8e4


# ---------------------------------------------------------------------------
# Workaround: this walrus build rejects >2 semaphore waits on one CTRL
# instruction; split the TileContext exit-drain waits onto individual nops.
def _patched_drain_and_barrier(self, tick_clock, wait_clock):
    nc = self.nc
    collect = nc.sync.nop()
    wait_clock.add_sem_waits(collect.ins, ScopedClock({None: tick_clock.global_clock}))
    si = collect.ins.sync_info
    waits = list(si.on_wait) if si and si.on_wait else []
    if len(waits) > 1:
        si.on_wait = waits[:1]
        for w in waits[1:]:
            nop = nc.sync.nop()
            if nop.ins.sync_info is None:
                nop.ins.sync_info = mybir.SyncInfo(on_wait=[w], on_update=[])
            else:
                nop.ins.sync_info.on_wait = [w]
    nc.sync.drain()
    nc.all_engine_barrier()
    popped = nc._tile_sem_poison_stack.pop()
    assert popped is self._sem_poison
    nc.clear_and_free_semaphores(list(self.sems.allocated().values()))
    nc.all_engine_barrier()


tile.TileContext._drain_and_barrier = _patched_drain_and_barrier


def _split_sync_waits(nc, max_waits=1):
    """Hoist excess semaphore waits (>max_waits per instruction) onto
    same-engine NoOp instructions inserted just before, preserving engine
    stream order (this walrus build encodes at most 2 waits/instruction)."""
    import bass_rust as _br

    ctr = [0]
    for fn in nc.m.functions:
        for bb in fn.blocks:
            out = []
            changed = False
            for inst in bb.instructions:
                si = getattr(inst, "sync_info", None)
                if si is not None and si.on_wait and len(si.on_wait) > max_waits:
                    waits = list(si.on_wait)
                    si.on_wait = waits[:max_waits]
                    rest = waits[max_waits:]
                    for j in range(0, len(rest), max_waits):
                        ctr[0] += 1
                        nop = _br.InstNoOp(name=f"WS-{ctr[0]}", ins=[], outs=[])
                        nop.engine = inst.engine
                        nop.sync_info = mybir.SyncInfo(
                            on_wait=rest[j : j + max_waits], on_update=[]
                        )
                        out.append(nop)
                    changed = True
                out.append(inst)
            if changed:
                bb.instructions = out


# ---------------------------------------------------------------------------


def _emit_lstm(nc, pools, T, d, xT_v, wih, whh, bias_sb, expand, hsT_v, tag):
    """Emit one LSTM direction's recurrence. Gates for step t are built in
    PSUM: a full-region bias-inject matmul (start=True — one accumulation
    group per step per the 2KB zero-region rule), then sum_k Wih[k] @ x[t]
    plus the two recurrent Whh tiles applied to h[t-1].

    T: sequence length. d: 0=forward, 1=backward (within this LSTM's hsT).
    xT_v:   [128, 3, T, BL] bf16 view of transposed embeddings (time-major).
    wih:    [128, 3, G4] bf16 input weights (lhsT tiles).
    whh:    [128, 2, G4] fp8 recurrent weights (lhsT tiles), host-scaled x2
            (hidden states are stored halved).
    bias_sb: [8, 128] bf16, row m = bias[m*128 : (m+1)*128].
    expand: [8, 128] bf16 constant, expand[k, m*16 + b] = (k == m).
    hsT_v:  [128, 4, BL, T] bf16 view; this direction writes slots 2d, 2d+1
            holding h/2 (downstream consumers compensate).
    """
    spool, pgates = pools
    CH = 4  # psum chunk: 4 timesteps per bank
    ctag = f"c_{tag}"
    steps = range(T - 1, -1, -1) if d == 1 else range(T)
    c_prev = None
    for si, t in enumerate(steps):
        tl = si % CH
        if tl == 0:
            g = pgates.tile([128, CH * 128], f32, tag=f"g_{tag}", name=f"g_{tag}_{si}")
        gv = g[:, tl * 128 : (tl + 1) * 128]
        gv3 = gv.rearrange("p (m b) -> p m b", b=BL)
        # bias inject over the whole step region opens the accumulation group
        nc.tensor.matmul(gv3, bias_sb, expand.rearrange("p (m b) -> p m b", b=BL),
                         start=True, stop=False)
        # input projection directly into PSUM (off the critical path),
        # then the recurrent matmuls accumulate on top (h[-1] = 0: skipped)
        for m in range(8):
            for k in range(3):
                # k=2 holds emb dims 192:320; the real rows 256:300 sit at
                # tile rows 64:108 (lhsT base partition must be 0/32/64)
                rows = slice(0, 128) if k < 2 else slice(64, 108)
                nc.tensor.matmul(
                    gv3[:, m, :],
                    wih[rows, k, m * 128 : (m + 1) * 128],
                    xT_v[rows, k, t, :],
                    start=False,
                    stop=(si == 0 and m == 7 and k == 2),
                )
        if si > 0:
            for m in range(8):
                nc.tensor.matmul(
                    gv3[:, m, :],
                    whh[:, :, m * 128 : (m + 1) * 128],
                    h8_prev,
                    start=False,
                    stop=(m == 7),
                    perf_mode=mybir.MatmulPerfMode.DoubleRow,
                )
        # all four gates through sigmoid in one op (g-gate pre-scaled by 2)
        sig = spool.tile([128, 128], bf16, tag=f"sig_{tag}", name=f"sig_{tag}_{si}")
        nc.scalar.activation(sig, gv, AF.Sigmoid)
        # c = sig(f)*c + sig(i)*tanh(g);  tanh(g) = 2*sig(2g)-1, so
        # mh := (sig(2g)-0.5)*sig(i) = sig(i)*tanh(g)/2
        cn = spool.tile([128, 32], bf16, tag=ctag, name=f"c_{tag}_{si}")
        mh = spool.tile([128, 32], bf16, tag=f"mh_{tag}", name=f"mh_{tag}_{si}")
        nc.vector.scalar_tensor_tensor(
            mh, sig[:, 64:96], -0.5, sig[:, 0:32], op0=ALU.add, op1=ALU.mult
        )
        if si == 0:
            nc.vector.tensor_scalar(cn, mh, 2.0, None, op0=ALU.mult)
        else:
            t2 = spool.tile([128, 32], bf16, tag=f"t2_{tag}", name=f"t2_{tag}_{si}")
            nc.vector.tensor_tensor(t2, sig[:, 32:64], c_prev, op=ALU.mult)
            nc.vector.scalar_tensor_tensor(cn, mh, 2.0, t2, op0=ALU.mult, op1=ALU.add)
        c_prev = cn
        # stored h/2 = (sig(2c)-0.5) * sig(o)  [= sig(o)*tanh(c)/2]
        # fp8 ring tile feeds the next step's DoubleRow matmuls; the bf16
        # hsT copy for attention/phase-3 happens off the critical chain
        vt = spool.tile([128, 32], bf16, tag=f"v_{tag}", name=f"v_{tag}_{si}")
        nc.scalar.activation(vt, cn, AF.Sigmoid, scale=2.0)
        h8 = spool.tile([128, 2, BL], f8, tag=f"h8_{tag}", name=f"h8_{tag}_{si}")
        nc.vector.scalar_tensor_tensor(
            h8,
            vt.rearrange("p (s b) -> p s b", b=BL), -0.5,
            sig[:, 96:128].rearrange("p (s b) -> p s b", b=BL),
            op0=ALU.add, op1=ALU.mult,
        )
        nc.vector.tensor_copy(hsT_v[:, 2 * d : 2 * d + 2, :, t], h8)
        h8_prev = h8
        yield


def _interleave(*gens, lead=0):
    """Drive per-step generators round-robin so the emitted instruction
    streams alternate between chains step by step (keeps the engine queues
    phase-balanced instead of relying on the scheduler's tie-breaks). The
    first generator is advanced `lead` extra steps up front to bias the two
    chains to a half-period phase offset."""
    alive = list(gens)
    for _ in range(lead):
        next(alive[0])
    while alive:
        nxt = []
        for gen in alive:
            try:
                next(gen)
                nxt.append(gen)
            except StopIteration:
                pass
        alive = nxt


def _build_program():
    nc = bass.Bass("TRN2", target_bir_lowering=False, debug=False)

    # --- DRAM I/O -----------------------------------------------------------
    d_emb = nc.dram_tensor("emb", [V, EP], bf16, kind="ExternalInput").ap()
    # col 0 = target chunk, cols 1..16 = sentence time chunks
    d_idx = nc.dram_tensor("idx", [128, 1 + NCH_S], dt.int32, kind="ExternalInput").ap()
    d_wih = {}
    d_whh = {}
    for nm in ("sf", "sb", "tf", "tb"):
        d_wih[nm] = nc.dram_tensor(f"wih_{nm}", [3, 128, G4], f8, kind="ExternalInput").ap()
        d_whh[nm] = nc.dram_tensor(f"whh_{nm}", [2, 128, G4], f8, kind="ExternalInput").ap()
    # cols 0:512 = biases (sf, sb, tf, tb), cols 512:640 = m-expansion matrix
    d_bx = nc.dram_tensor("bx", [8, 640], bf16, kind="ExternalInput").ap()
    d_wout = nc.dram_tensor("woutT", [4, 128, OUT], bf16, kind="ExternalInput").ap()
    d_bout = nc.dram_tensor("boutT", [OUT, 1], f32, kind="ExternalInput").ap()
    d_out = nc.dram_tensor("out", [BL, OUT], f32, kind="ExternalOutput").ap()

    with tile.TileContext(nc) as tc:
        with (
            tc.tile_pool(name="cpool", bufs=1) as cpool,
            tc.tile_pool(name="spool", bufs=3) as spool,
            tc.tile_pool(name="gpool", bufs=6) as gpool,
            tc.tile_pool(name="pgates", bufs=2, space="PSUM") as pgates,
        ):
            # --- constants / weights into SBUF ------------------------------
            # index + bias loads first on SP (gathers depend on idx); big
            # weight loads split across the ACT and DVE DMA queues so the
            # first recurrence steps aren't starved behind a serial SP queue.
            idx = cpool.tile([128, 1 + NCH_S], dt.int32, name="idx")
            nc.sync.dma_start(idx, d_idx)
            bx = cpool.tile([8, 640], bf16, name="bx")
            nc.sync.dma_start(bx, d_bx)
            wih = {}
            whh = {}
            bias_sb = {}
            for nm, eng in (("tf", nc.sync), ("tb", nc.scalar), ("sf", nc.sync), ("sb", nc.scalar)):
                wt = cpool.tile([128, 3, G4], f8, name=f"wih_{nm}_sb")
                eng.dma_start(wt, d_wih[nm].rearrange("k p m -> p k m"))
                wih[nm] = wt
                ht = cpool.tile([128, 2, G4], f8, name=f"whh_{nm}_sb")
                eng.dma_start(ht, d_whh[nm].rearrange("k p m -> p k m"))
                whh[nm] = ht
            for i, nm in enumerate(("sf", "sb", "tf", "tb")):
                bias_sb[nm] = bx[:, 128 * i : 128 * (i + 1)]
            expand = bx[:, 512:640]
            # preload the sigmoid ACT table after the ACT-queue weight DMAs
            # (ready before the first real sig, without delaying the loads)
            scr = cpool.tile([1, 1], f32, name="scr")
            nc.vector.memset(scr, 0.0)
            nc.scalar.activation(scr, scr, AF.Sigmoid)
            wout_sb = cpool.tile([128, 4, OUT], bf16, name="wout_sb")
            nc.sync.dma_start(wout_sb, d_wout.rearrange("k p m -> p k m"))
            bout_sb = cpool.tile([OUT, 1], f32, name="bout_sb")
            nc.sync.dma_start(bout_sb, d_bout)

            ident = cpool.tile([128, 128], f32, name="ident")
            make_identity(nc, ident)
            ones = cpool.tile([128, 128], f32, name="ones")
            nc.gpsimd.memset(ones, 1.0)

            # --- persistent activations -------------------------------------
            xT_s = cpool.tile([128, 3, NTOK_S], bf16, name="xT_s")
            xT_t = cpool.tile([128, 3, NTOK_T], bf16, name="xT_t")
            hsT = cpool.tile([128, 4, NTOK_S], bf16, name="hsT")
            ttT = cpool.tile([128, 4, NTOK_T], bf16, name="ttT")

            # --- phase 1: gather + DMA transpose (time-major chunks) --------
            # xT free layout is time-major (col = t*BL + b), so chunk j of
            # the sentence (timesteps [8j, 8j+8) for all 16 batch elems) is a
            # contiguous 128-col block and every DMA-transpose output is a
            # plain 2D [128, 128] tile. fwd starts after chunk 0, bwd after
            # chunk 15, so the gathers are ordered from both ends inward.
            def gather_chunk(col, j, which):
                gx = gpool.tile([128, EP], bf16, tag="gx", name=f"gx_{which}_{j}")
                nc.gpsimd.indirect_dma_start(
                    out=gx,
                    out_offset=None,
                    in_=d_emb[:, :],
                    in_offset=bass.IndirectOffsetOnAxis(ap=idx[:, col : col + 1], axis=0),
                )
                for k in range(3):
                    if which == "s":
                        out = xT_s[:, k, j * 128 : (j + 1) * 128]
                    else:
                        out = xT_t[:, k, :]
                    lo = k * 128 if k < 2 else 192
                    nc.sync.dma_start_transpose(out, gx[:, lo : lo + 128])

            order = []
            lo, hi = 0, NCH_S - 1
            while lo <= hi:
                order.append(lo)
                if hi != lo:
                    order.append(hi)
                lo += 1
                hi -= 1
            gather_chunk(0, 0, "t")
            for j in order:
                gather_chunk(1 + j, j, "s")

            # --- phase 2: recurrences ---------------------------------------
            lpools = (spool, pgates)
            ttT_v = ttT.rearrange("p s (b t) -> p s b t", b=BL)
            xTt_v = xT_t.rearrange("p k (t b) -> p k t b", b=BL)
            xTs_v = xT_s.rearrange("p k (t b) -> p k t b", b=BL)
            hsT_v = hsT.rearrange("p s (b t) -> p s b t", b=BL)
            _interleave(
                _emit_lstm(nc, lpools, LT, 0, xTt_v, wih["tf"], whh["tf"], bias_sb["tf"], expand, ttT_v, "tf"),
                _emit_lstm(nc, lpools, LT, 1, xTt_v, wih["tb"], whh["tb"], bias_sb["tb"], expand, ttT_v, "tb"),
            )
            _interleave(
                _emit_lstm(nc, lpools, LS, 0, xTs_v, wih["sf"], whh["sf"], bias_sb["sf"], expand, hsT_v, "sf"),
                _emit_lstm(nc, lpools, LS, 1, xTs_v, wih["sb"], whh["sb"], bias_sb["sb"], expand, hsT_v, "sb"),
                lead=1,
            )

        # --- phase 3: attention + output head ------------------------------
        with (
            tc.tile_pool(name="apool", bufs=1) as apool,
            tc.tile_pool(name="patt", bufs=1, space="PSUM") as patt,
        ):
            # A[b,s,t] stored as [s(part), b*8+t]
            a3 = patt.tile([128, 128], f32, name="a3")
            for b in range(BL):
                for k in range(4):
                    nc.tensor.matmul(
                        a3[:, b * 8 : (b + 1) * 8],
                        hsT[:, k, b * 128 : (b + 1) * 128],
                        ttT[:, k, b * 8 : (b + 1) * 8],
                        start=(k == 0),
                        stop=(k == 3),
                    )
            expA = apool.tile([128, 128], f32, name="expA")
            nc.scalar.activation(expA, a3, AF.Exp, scale=4.0)
            expA_v = expA.rearrange("p (b t) -> p b t", t=LT)
            # row softmax (over t) then mean over s, divided by col sums (over s)
            rsum = apool.tile([128, BL], f32, name="rsum")
            nc.vector.tensor_reduce(rsum, expA_v, axis=mybir.AxisListType.X, op=ALU.add)
            rr = apool.tile([128, BL], f32, name="rr")
            nc.vector.reciprocal(rr, rsum)
            rnorm = apool.tile([128, 128], f32, name="rnorm")
            rr_b = bass.AP(tensor=rr.tensor, offset=rr.offset, ap=list(rr.ap) + [[0, LT]])
            nc.vector.tensor_tensor(rnorm.rearrange("p (b t) -> p b t", t=LT), expA_v, rr_b, op=ALU.mult)
            rvp = patt.tile([1, 128], f32, name="rvp")
            nc.tensor.matmul(rvp, ones[:, 0:1], rnorm, start=True, stop=True)
            csum = patt.tile([1, 128], f32, name="csum")
            nc.tensor.matmul(csum, ones[:, 0:1], expA, start=True, stop=True)
            rc = apool.tile([1, 128], f32, name="rc")
            nc.vector.reciprocal(rc, csum)
            q = apool.tile([1, 128], f32, name="q")
            nc.vector.scalar_tensor_tensor(q, rvp, 1.0 / LS, rc, op0=ALU.mult, op1=ALU.mult)
            qbc = patt.tile([128, 128], f32, name="qbc")
            nc.tensor.matmul(qbc, ones[0:1, :], q, start=True, stop=True)
            attw = apool.tile([128, 128], f32, name="attw")
            nc.vector.tensor_tensor(attw, expA, qbc, op=ALU.mult)
            attnT = apool.tile([128, BL], f32, name="attnT")
            nc.vector.tensor_reduce(attnT, attw.rearrange("p (b t) -> p b t", t=LT), axis=mybir.AxisListType.X, op=ALU.add)
            attnb = apool.tile([128, BL], bf16, name="attnb")
            nc.vector.tensor_copy(attnb, attnT)

            # de-transpose sen_h for the score contraction; issue cost is
            # ~112ns each, so split the 64 transposes across both DMA queues
            sen_h = apool.tile([128, BL, 4 * 128], bf16, name="sen_h")
            for b in range(BL):
                for k in range(4):
                    eng = nc.sync if (b * 4 + k) % 2 == 0 else nc.scalar
                    eng.dma_start_transpose(
                        sen_h[:, b, k * 128 : (k + 1) * 128], hsT[:, k, b * 128 : (b + 1) * 128]
                    )
            scoT = patt.tile([128, 4 * BL], f32, name="scoT")
            for b in range(BL):
                for mh in range(4):
                    nc.tensor.matmul(
                        scoT[:, b * 4 + mh : b * 4 + mh + 1],
                        sen_h[:, b, mh * 128 : (mh + 1) * 128],
                        attnb[:, b : b + 1],
                        start=True,
                        stop=True,
                    )
            scoB = apool.tile([128, 4 * BL], bf16, name="scoB")
            nc.scalar.activation(scoB, scoT, AF.Copy)
            lgT = patt.tile([OUT, BL], f32, name="lgT")
            for mh in range(4):
                nc.tensor.matmul(
                    lgT, wout_sb[:, mh, :], scoB[:, mh :: 4], start=(mh == 0), stop=(mh == 3)
                )
            lgsb = apool.tile([OUT, BL], f32, name="lgsb")
            nc.scalar.activation(lgsb, lgT, AF.Identity, bias=bout_sb[0:OUT, 0:1])
            lg2 = patt.tile([BL, OUT], f32, name="lg2")
            nc.tensor.transpose(lg2, lgsb, ident[0:OUT, 0:OUT])
            eo = apool.tile([BL, OUT], f32, name="eo")
            nc.scalar.activation(eo, lg2, AF.Exp)
            es = apool.tile([BL, 1], f32, name="es")
            nc.vector.tensor_reduce(es, eo, axis=mybir.AxisListType.X, op=ALU.add)
            er = apool.tile([BL, 1], f32, name="er")
            nc.vector.reciprocal(er, es)
            res = apool.tile([BL, OUT], f32, name="res")
            nc.vector.tensor_scalar(res, eo, er, None, op0=ALU.mult)
            nc.sync.dma_start(d_out, res)

    _split_sync_waits(nc)
    return nc


_CACHE = {}


def _get_program():
    if "nc" not in _CACHE:
        _CACHE["nc"] = _build_program()
    return _CACHE["nc"]


def prepare_in_maps(inputs):
    """Host-side prep: shard + repack inputs into per-core in_maps."""
    bf = ml_dtypes.bfloat16
    sen = np.asarray(inputs["sentence_source"]).astype(np.int32)  # [B, LS]
    tgt = np.asarray(inputs["target_source"]).astype(np.int32)  # [B, LT]
    emb = np.asarray(inputs["emb_W"], dtype=np.float32).copy()
    emb[0, :] = 0.0  # padding_idx
    emb_bf = np.zeros((V, EP), dtype=bf)
    emb_bf[:, :E] = emb.astype(bf)
    emb_bf[:, E] = 1.0  # multiplies the bias row of wih (k=2, row 44)

    def pack_wih(nm):
        W = np.asarray(inputs[f"Wih_{nm}"], dtype=np.float32).T.copy()  # [300, 1024]
        W[:, 2 * H : 3 * H] *= 2.0  # g-gate: tanh via sigmoid
        e4 = ml_dtypes.float8_e4m3
        pack = np.zeros((3, 128, G4), dtype=e4)
        pack[0] = W[0:128].astype(e4)
        pack[1] = W[128:256].astype(e4)
        pack[2, 64:108] = W[256:300].astype(e4)
        return pack

    def pack_bias(nm):
        bias = (
            np.asarray(inputs[f"bih_{nm}"], dtype=np.float32)
            + np.asarray(inputs[f"bhh_{nm}"], dtype=np.float32)
        ).copy()
        bias[2 * H : 3 * H] *= 2.0
        return np.ascontiguousarray(bias.reshape(8, 128).astype(bf))

    def pack_whh(nm):
        W = np.asarray(inputs[f"Whh_{nm}"], dtype=np.float32).T.copy()  # [256, 1024]
        W *= 2.0  # hidden states are stored halved
        W[:, 2 * H : 3 * H] *= 2.0
        return np.ascontiguousarray(W.reshape(2, 128, G4).astype(ml_dtypes.float8_e4m3))

    shared = {"emb": emb_bf}
    bx = np.zeros((8, 640), dtype=bf)
    for i, nm in enumerate(("sf", "sb", "tf", "tb")):
        shared[f"wih_{nm}"] = pack_wih(nm)
        shared[f"whh_{nm}"] = pack_whh(nm)
        bx[:, 128 * i : 128 * (i + 1)] = pack_bias(nm)
    bx[:, 512:640] = np.repeat(np.eye(8), BL, axis=1).astype(bf)
    shared["bx"] = np.ascontiguousarray(bx)
    Wout = np.asarray(inputs["Wout"], dtype=np.float32) * 2.0  # [3, 512]; sen_h halved
    shared["woutT"] = np.ascontiguousarray(Wout.T.reshape(4, 128, OUT).astype(bf))
    shared["boutT"] = np.asarray(inputs["bout"], dtype=np.float32).reshape(OUT, 1)

    in_maps = []
    for c in range(NCORES):
        sl = slice(c * BL, (c + 1) * BL)
        m = dict(shared)
        # sentence chunk j = timesteps [8j, 8j+8) across all 16 batch elems,
        # time-major within the chunk (matches the xT free layout t*16+b)
        sh = sen[sl]  # [16, 128]
        cols = [tgt[sl].T.reshape(-1)]
        cols += [sh[:, TS_CH * j : TS_CH * (j + 1)].T.reshape(-1) for j in range(NCH_S)]
        m["idx"] = np.ascontiguousarray(np.stack(cols, axis=1))  # [128, 17]
        in_maps.append(m)
    return in_maps


def kernel(**inputs) -> np.ndarray:
    nc = _get_program()
    in_maps = prepare_in_maps(inputs)
    r = run_bass_kernel_spmd(nc, in_maps, core_ids=list(range(NCORES)))
    return np.concatenate([r.results[c]["out"] for c in range(NCORES)], axis=0)


if __name__ == "__main__":
    print("building program...")
    nc = _get_program()
    print("build OK")


# revision 57
# speedup vs baseline: 7.4126x; 2.9672x over previous
"""Trainium2 Bass kernel for nn_Encoder (bidirectional-LSTM encoder + attention).

Strategy: data-parallel over batch B=128 across 8 cores (16 batch elems/core).
Each core runs the full pipeline locally (embedding gather, both LSTM
directions for sentence+target, attention, output head). No cross-core
communication; host concatenates the per-core [16, 3] outputs.

v3: the input projections are folded into the recurrence itself (3 extra
matmuls per step accumulate Wih@x + bias into the gate PSUM before the Whh
matmuls), the embedding table is gathered in bf16 and transposed by the DMA
xbar instead of the PE, and gathers are chunked time-major so all four LSTM
chains (sen fwd/bwd, tgt fwd/bwd) start as soon as their first/last time
chunks land. All LSTM state is gate-transposed ([gate_dim, batch]); tanh is
computed via sigmoid (tanh(x) = 2*sigmoid(2x) - 1, with g-gate weight rows
pre-scaled by 2 on the host) so the ACT table never switches mid-recurrence.
"""

import sys

sys.path.insert(0, "/opt/trn_rl_repo")

import numpy as np
import ml_dtypes

import concourse.bass as bass
import concourse.mybir as mybir
import concourse.tile as tile
from concourse.bass_utils import run_bass_kernel_spmd
from concourse.masks import make_identity
from concourse.vector_clock import ScopedClock

V, E, H, OUT = 100000, 300, 256, 3
B, LS, LT = 128, 128, 8
NCORES = 8
BL = B // NCORES  # 16 batch elements per core
G4 = 4 * H  # 1024 (gate dim)
NTOK_S = BL * LS  # 2048 sentence tokens per core
NTOK_T = BL * LT  # 128 target tokens per core
NCH_S = NTOK_S // 128  # 16 time-major chunks, 8 timesteps each
TS_CH = LS // NCH_S  # 8 timesteps per sentence chunk
EP = 320  # embedding row padded; k-slices 0:128, 128:256, 192:320 (overlap)

dt = mybir.dt
AF = mybir.ActivationFunctionType
ALU = mybir.AluOpType
f32 = dt.float32
bf16 = dt.bfloat16
f8 = dt.float8e4


# ---------------------------------------------------------------------------
# Workaround: this walrus build rejects >2 semaphore waits on one CTRL
# instruction; split the TileContext exit-drain waits onto individual nops.
def _patched_drain_and_barrier(self, tick_clock, wait_clock):
    nc = self.nc
    collect = nc.sync.nop()
    wait_clock.add_sem_waits(collect.ins, ScopedClock({None: tick_clock.global_clock}))
    si = collect.ins.sync_info
    waits = list(si.on_wait) if si and si.on_wait else []
    if len(waits) > 1:
        si.on_wait = waits[:1]
        for w in waits[1:]:
            nop = nc.sync.nop()
            if nop.ins.sync_info is None:
                nop.ins.sync_info = mybir.SyncInfo(on_wait=[w], on_update=[])
            else:
                nop.ins.sync_info.on_wait = [w]
    nc.sync.drain()
    nc.all_engine_barrier()
    popped = nc._tile_sem_poison_stack.pop()
    assert popped is self._sem_poison
    nc.clear_and_free_semaphores(list(self.sems.allocated().values()))
    nc.all_engine_barrier()


tile.TileContext._drain_and_barrier = _patched_drain_and_barrier


def _split_sync_waits(nc, max_waits=1):
    """Hoist excess semaphore waits (>max_waits per instruction) onto
    same-engine NoOp instructions inserted just before, preserving engine
    stream order (this walrus build encodes at most 2 waits/instruction)."""
    import bass_rust as _br

    ctr = [0]
    for fn in nc.m.functions:
        for bb in fn.blocks:
            out = []
            changed = False
            for inst in bb.instructions:
                si = getattr(inst, "sync_info", None)
                if si is not None and si.on_wait and len(si.on_wait) > max_waits:
                    waits = list(si.on_wait)
                    si.on_wait = waits[:max_waits]
                    rest = waits[max_waits:]
                    for j in range(0, len(rest), max_waits):
                        ctr[0] += 1
                        nop = _br.InstNoOp(name=f"WS-{ctr[0]}", ins=[], outs=[])
                        nop.engine = inst.engine
                        nop.sync_info = mybir.SyncInfo(
                            on_wait=rest[j : j + max_waits], on_update=[]
                        )
                        out.append(nop)
                    changed = True
                out.append(inst)
            if changed:
                bb.instructions = out


# ---------------------------------------------------------------------------


def _emit_lstm(nc, pools, T, d, xT_v, wih, whh, bias_sb, expand, hsT_v, tag):
    """Emit one LSTM direction's recurrence. Gates for step t are built in
    PSUM: a full-region bias-inject matmul (start=True — one accumulation
    group per step per the 2KB zero-region rule), then sum_k Wih[k] @ x[t]
    plus the two recurrent Whh tiles applied to h[t-1].

    T: sequence length. d: 0=forward, 1=backward (within this LSTM's hsT).
    xT_v:   [128, 3, T, BL] bf16 view of transposed embeddings (time-major).
    wih:    [128, 3, G4] bf16 input weights (lhsT tiles).
    whh:    [128, 2, G4] fp8 recurrent weights (lhsT tiles), host-scaled x2
            (hidden states are stored halved).
    bias_sb: [8, 128] bf16, row m = bias[m*128 : (m+1)*128].
    expand: [8, 128] bf16 constant, expand[k, m*16 + b] = (k == m).
    hsT_v:  [128, 4, BL, T] bf16 view; this direction writes slots 2d, 2d+1
            holding h/2 (downstream consumers compensate).
    """
    spool, pgates = pools
    CH = 4  # psum chunk: 4 timesteps per bank
    ctag = f"c_{tag}"
    steps = range(T - 1, -1, -1) if d == 1 else range(T)
    c_prev = None
    for si, t in enumerate(steps):
        tl = si % CH
        if tl == 0:
            g = pgates.tile([128, CH * 128], f32, tag=f"g_{tag}", name=f"g_{tag}_{si}")
        gv = g[:, tl * 128 : (tl + 1) * 128]
        gv3 = gv.rearrange("p (m b) -> p m b", b=BL)
        # bias inject over the whole step region opens the accumulation group
        nc.tensor.matmul(gv3, bias_sb, expand.rearrange("p (m b) -> p m b", b=BL),
                         start=True, stop=False)
        # input projection directly into PSUM (off the critical path),
        # then the recurrent matmuls accumulate on top (h[-1] = 0: skipped)
        for m in range(8):
            for k in range(3):
                # k=2 holds emb dims 192:320; the real rows 256:300 sit at
                # tile rows 64:108 (lhsT base partition must be 0/32/64)
                rows = slice(0, 128) if k < 2 else slice(64, 108)
                nc.tensor.matmul(
                    gv3[:, m, :],
                    wih[rows, k, m * 128 : (m + 1) * 128],
                    xT_v[rows, k, t, :],
                    start=False,
                    stop=(si == 0 and m == 7 and k == 2),
                )
        if si > 0:
            for m in range(8):
                nc.tensor.matmul(
                    gv3[:, m, :],
                    whh[:, :, m * 128 : (m + 1) * 128],
                    h8_prev,
                    start=False,
                    stop=(m == 7),
                    perf_mode=mybir.MatmulPerfMode.DoubleRow,
                )
        # all four gates through sigmoid in one op (g-gate pre-scaled by 2)
        sig = spool.tile([128, 128], bf16, tag=f"sig_{tag}", name=f"sig_{tag}_{si}")
        nc.scalar.activation(sig, gv, AF.Sigmoid)
        # cell state tracked halved (C := c/2) so the update is scalar-free:
        # C = sig(f)*C + mh with mh := (sig(2g)-0.5)*sig(i) = sig(i)*tanh(g)/2
        # (plain tensor_tensor ops get the DVE 2x bf16 mode; STT does not)
        cn = spool.tile([128, 32], bf16, tag=ctag, name=f"c_{tag}_{si}")
        mh = spool.tile([128, 32], bf16, tag=f"mh_{tag}", name=f"mh_{tag}_{si}")
        nc.vector.scalar_tensor_tensor(
            mh, sig[:, 64:96], -0.5, sig[:, 0:32], op0=ALU.add, op1=ALU.mult
        )
        if si == 0:
            nc.vector.tensor_copy(cn, mh)
        else:
            t2 = spool.tile([128, 32], bf16, tag=f"t2_{tag}", name=f"t2_{tag}_{si}")
            nc.vector.tensor_tensor(t2, sig[:, 32:64], c_prev, op=ALU.mult)
            nc.vector.tensor_tensor(cn, mh, t2, op=ALU.add)
        c_prev = cn
        # stored h/2 = (sig(4C)-0.5) * sig(o)  [= sig(o)*tanh(c)/2]
        # fp8 ring tile feeds the next step's DoubleRow matmuls; the bf16
        # hsT copy for attention/phase-3 happens off the critical chain
        vt = spool.tile([128, 32], bf16, tag=f"v_{tag}", name=f"v_{tag}_{si}")
        nc.scalar.activation(vt, cn, AF.Sigmoid, scale=4.0)
        h8 = spool.tile([128, 2, BL], f8, tag=f"h8_{tag}", name=f"h8_{tag}_{si}")
        nc.vector.scalar_tensor_tensor(
            h8,
            vt.rearrange("p (s b) -> p s b", b=BL), -0.5,
            sig[:, 96:128].rearrange("p (s b) -> p s b", b=BL),
            op0=ALU.add, op1=ALU.mult,
        )
        nc.vector.tensor_copy(hsT_v[:, 2 * d : 2 * d + 2, :, t], h8)
        h8_prev = h8
        yield


def _interleave(*gens, lead=0):
    """Drive per-step generators round-robin so the emitted instruction
    streams alternate between chains step by step (keeps the engine queues
    phase-balanced instead of relying on the scheduler's tie-breaks). The
    first generator is advanced `lead` extra steps up front to bias the two
    chains to a half-period phase offset."""
    alive = list(gens)
    for _ in range(lead):
        next(alive[0])
    while alive:
        nxt = []
        for gen in alive:
            try:
                next(gen)
                nxt.append(gen)
            except StopIteration:
                pass
        alive = nxt


def _build_program():
    nc = bass.Bass("TRN2", target_bir_lowering=False, debug=False)

    # --- DRAM I/O -----------------------------------------------------------
    d_emb = nc.dram_tensor("emb", [V, EP], bf16, kind="ExternalInput").ap()
    # col 0 = target chunk, cols 1..16 = sentence time chunks
    d_idx = nc.dram_tensor("idx", [128, 1 + NCH_S], dt.int32, kind="ExternalInput").ap()
    d_wih = {}
    d_whh = {}
    for nm in ("sf", "sb", "tf", "tb"):
        d_wih[nm] = nc.dram_tensor(f"wih_{nm}", [3, 128, G4], f8, kind="ExternalInput").ap()
        d_whh[nm] = nc.dram_tensor(f"whh_{nm}", [2, 128, G4], f8, kind="ExternalInput").ap()
    # cols 0:512 = biases (sf, sb, tf, tb), cols 512:640 = m-expansion matrix
    d_bx = nc.dram_tensor("bx", [8, 640], bf16, kind="ExternalInput").ap()
    d_wout = nc.dram_tensor("woutT", [4, 128, OUT], bf16, kind="ExternalInput").ap()
    d_bout = nc.dram_tensor("boutT", [OUT, 1], f32, kind="ExternalInput").ap()
    d_out = nc.dram_tensor("out", [BL, OUT], f32, kind="ExternalOutput").ap()

    with tile.TileContext(nc) as tc:
        with (
            tc.tile_pool(name="cpool", bufs=1) as cpool,
            tc.tile_pool(name="spool", bufs=3) as spool,
            tc.tile_pool(name="gpool", bufs=6) as gpool,
            tc.tile_pool(name="pgates", bufs=2, space="PSUM") as pgates,
        ):
            # --- constants / weights into SBUF ------------------------------
            # index + bias loads first on SP (gathers depend on idx); big
            # weight loads split across the ACT and DVE DMA queues so the
            # first recurrence steps aren't starved behind a serial SP queue.
            idx = cpool.tile([128, 1 + NCH_S], dt.int32, name="idx")
            nc.sync.dma_start(idx, d_idx)
            bx = cpool.tile([8, 640], bf16, name="bx")
            nc.sync.dma_start(bx, d_bx)
            wih = {}
            whh = {}
            bias_sb = {}
            for nm, eng in (("tf", nc.sync), ("tb", nc.scalar), ("sf", nc.sync), ("sb", nc.scalar)):
                wt = cpool.tile([128, 3, G4], f8, name=f"wih_{nm}_sb")
                eng.dma_start(wt, d_wih[nm].rearrange("k p m -> p k m"))
                wih[nm] = wt
                ht = cpool.tile([128, 2, G4], f8, name=f"whh_{nm}_sb")
                eng.dma_start(ht, d_whh[nm].rearrange("k p m -> p k m"))
                whh[nm] = ht
            for i, nm in enumerate(("sf", "sb", "tf", "tb")):
                bias_sb[nm] = bx[:, 128 * i : 128 * (i + 1)]
            expand = bx[:, 512:640]
            # preload the sigmoid ACT table after the ACT-queue weight DMAs
            # (ready before the first real sig, without delaying the loads)
            scr = cpool.tile([1, 1], f32, name="scr")
            nc.vector.memset(scr, 0.0)
            nc.scalar.activation(scr, scr, AF.Sigmoid)
            wout_sb = cpool.tile([128, 4, OUT], bf16, name="wout_sb")
            nc.sync.dma_start(wout_sb, d_wout.rearrange("k p m -> p k m"))
            bout_sb = cpool.tile([OUT, 1], f32, name="bout_sb")
            nc.sync.dma_start(bout_sb, d_bout)

            ident = cpool.tile([128, 128], f32, name="ident")
            make_identity(nc, ident)
            ones = cpool.tile([128, 128], f32, name="ones")
            nc.gpsimd.memset(ones, 1.0)

            # --- persistent activations -------------------------------------
            xT_s = cpool.tile([128, 3, NTOK_S], bf16, name="xT_s")
            xT_t = cpool.tile([128, 3, NTOK_T], bf16, name="xT_t")
            hsT = cpool.tile([128, 4, NTOK_S], bf16, name="hsT")
            ttT = cpool.tile([128, 4, NTOK_T], bf16, name="ttT")

            # --- phase 1: gather + DMA transpose (time-major chunks) --------
            # xT free layout is time-major (col = t*BL + b), so chunk j of
            # the sentence (timesteps [8j, 8j+8) for all 16 batch elems) is a
            # contiguous 128-col block and every DMA-transpose output is a
            # plain 2D [128, 128] tile. fwd starts after chunk 0, bwd after
            # chunk 15, so the gathers are ordered from both ends inward.
            def gather_chunk(col, j, which, split=False):
                gx = gpool.tile([128, EP], bf16, tag="gx", name=f"gx_{which}_{j}")
                # first chunks gate the recurrence start: split them into two
                # 64-row gathers on separate SWDGE queues to halve the latency
                parts = ((0, 64), (64, 128)) if split else ((0, 128),)
                for lo, hi in parts:
                    nc.gpsimd.indirect_dma_start(
                        out=gx[lo:hi, :],
                        out_offset=None,
                        in_=d_emb[:, :],
                        in_offset=bass.IndirectOffsetOnAxis(ap=idx[lo:hi, col : col + 1], axis=0),
                    )
                for k in range(3):
                    if which == "s":
                        out = xT_s[:, k, j * 128 : (j + 1) * 128]
                    else:
                        out = xT_t[:, k, :]
                    lo = k * 128 if k < 2 else 192
                    nc.sync.dma_start_transpose(out, gx[:, lo : lo + 128])

            order = []
            lo, hi = 0, NCH_S - 1
            while lo <= hi:
                order.append(lo)
                if hi != lo:
                    order.append(hi)
                lo += 1
                hi -= 1
            gather_chunk(0, 0, "t")
            for j in order:
                gather_chunk(1 + j, j, "s")

            # --- phase 2: recurrences ---------------------------------------
            lpools = (spool, pgates)
            ttT_v = ttT.rearrange("p s (b t) -> p s b t", b=BL)
            xTt_v = xT_t.rearrange("p k (t b) -> p k t b", b=BL)
            xTs_v = xT_s.rearrange("p k (t b) -> p k t b", b=BL)
            hsT_v = hsT.rearrange("p s (b t) -> p s b t", b=BL)
            _interleave(
                _emit_lstm(nc, lpools, LT, 0, xTt_v, wih["tf"], whh["tf"], bias_sb["tf"], expand, ttT_v, "tf"),
                _emit_lstm(nc, lpools, LT, 1, xTt_v, wih["tb"], whh["tb"], bias_sb["tb"], expand, ttT_v, "tb"),
            )
            _interleave(
                _emit_lstm(nc, lpools, LS, 0, xTs_v, wih["sf"], whh["sf"], bias_sb["sf"], expand, hsT_v, "sf"),
                _emit_lstm(nc, lpools, LS, 1, xTs_v, wih["sb"], whh["sb"], bias_sb["sb"], expand, hsT_v, "sb"),
                lead=1,
            )

        # --- phase 3: attention + output head ------------------------------
        with (
            tc.tile_pool(name="apool", bufs=1) as apool,
            tc.tile_pool(name="patt", bufs=1, space="PSUM") as patt,
        ):
            # A[b,s,t] stored as [s(part), b*8+t]
            a3 = patt.tile([128, 128], f32, name="a3")
            for b in range(BL):
                for k in range(4):
                    nc.tensor.matmul(
                        a3[:, b * 8 : (b + 1) * 8],
                        hsT[:, k, b * 128 : (b + 1) * 128],
                        ttT[:, k, b * 8 : (b + 1) * 8],
                        start=(k == 0),
                        stop=(k == 3),
                    )
            expA = apool.tile([128, 128], f32, name="expA")
            nc.scalar.activation(expA, a3, AF.Exp, scale=4.0)
            expA_v = expA.rearrange("p (b t) -> p b t", t=LT)
            # row softmax (over t) then mean over s, divided by col sums (over s)
            rsum = apool.tile([128, BL], f32, name="rsum")
            nc.vector.tensor_reduce(rsum, expA_v, axis=mybir.AxisListType.X, op=ALU.add)
            rr = apool.tile([128, BL], f32, name="rr")
            nc.vector.reciprocal(rr, rsum)
            rnorm = apool.tile([128, 128], f32, name="rnorm")
            rr_b = bass.AP(tensor=rr.tensor, offset=rr.offset, ap=list(rr.ap) + [[0, LT]])
            nc.vector.tensor_tensor(rnorm.rearrange("p (b t) -> p b t", t=LT), expA_v, rr_b, op=ALU.mult)
            rvp = patt.tile([1, 128], f32, name="rvp")
            nc.tensor.matmul(rvp, ones[:, 0:1], rnorm, start=True, stop=True)
            csum = patt.tile([1, 128], f32, name="csum")
            nc.tensor.matmul(csum, ones[:, 0:1], expA, start=True, stop=True)
            rc = apool.tile([1, 128], f32, name="rc")
            nc.vector.reciprocal(rc, csum)
            q = apool.tile([1, 128], f32, name="q")
            nc.vector.scalar_tensor_tensor(q, rvp, 1.0 / LS, rc, op0=ALU.mult, op1=ALU.mult)
            qbc = patt.tile([128, 128], f32, name="qbc")
            nc.tensor.matmul(qbc, ones[0:1, :], q, start=True, stop=True)
            attw = apool.tile([128, 128], f32, name="attw")
            nc.vector.tensor_tensor(attw, expA, qbc, op=ALU.mult)
            attnT = apool.tile([128, BL], f32, name="attnT")
            nc.vector.tensor_reduce(attnT, attw.rearrange("p (b t) -> p b t", t=LT), axis=mybir.AxisListType.X, op=ALU.add)
            attnb = apool.tile([128, BL], bf16, name="attnb")
            nc.vector.tensor_copy(attnb, attnT)

            # de-transpose sen_h for the score contraction; issue cost is
            # ~112ns each, so split the 64 transposes across both DMA queues
            sen_h = apool.tile([128, BL, 4 * 128], bf16, name="sen_h")
            for b in range(BL):
                for k in range(4):
                    eng = nc.sync if (b * 4 + k) % 2 == 0 else nc.scalar
                    eng.dma_start_transpose(
                        sen_h[:, b, k * 128 : (k + 1) * 128], hsT[:, k, b * 128 : (b + 1) * 128]
                    )
            scoT = patt.tile([128, 4 * BL], f32, name="scoT")
            for b in range(BL):
                for mh in range(4):
                    nc.tensor.matmul(
                        scoT[:, b * 4 + mh : b * 4 + mh + 1],
                        sen_h[:, b, mh * 128 : (mh + 1) * 128],
                        attnb[:, b : b + 1],
                        start=True,
                        stop=True,
                    )
            scoB = apool.tile([128, 4 * BL], bf16, name="scoB")
            nc.scalar.activation(scoB, scoT, AF.Copy)
            lgT = patt.tile([OUT, BL], f32, name="lgT")
            for mh in range(4):
                nc.tensor.matmul(
                    lgT, wout_sb[:, mh, :], scoB[:, mh :: 4], start=(mh == 0), stop=(mh == 3)
                )
            lgsb = apool.tile([OUT, BL], f32, name="lgsb")
            nc.scalar.activation(lgsb, lgT, AF.Identity, bias=bout_sb[0:OUT, 0:1])
            lg2 = patt.tile([BL, OUT], f32, name="lg2")
            nc.tensor.transpose(lg2, lgsb, ident[0:OUT, 0:OUT])
            eo = apool.tile([BL, OUT], f32, name="eo")
            nc.scalar.activation(eo, lg2, AF.Exp)
            es = apool.tile([BL, 1], f32, name="es")
            nc.vector.tensor_reduce(es, eo, axis=mybir.AxisListType.X, op=ALU.add)
            er = apool.tile([BL, 1], f32, name="er")
            nc.vector.reciprocal(er, es)
            res = apool.tile([BL, OUT], f32, name="res")
            nc.vector.tensor_scalar(res, eo, er, None, op0=ALU.mult)
            nc.sync.dma_start(d_out, res)

    _split_sync_waits(nc)
    return nc


_CACHE = {}


def _get_program():
    if "nc" not in _CACHE:
        _CACHE["nc"] = _build_program()
    return _CACHE["nc"]


def prepare_in_maps(inputs):
    """Host-side prep: shard + repack inputs into per-core in_maps."""
    bf = ml_dtypes.bfloat16
    sen = np.asarray(inputs["sentence_source"]).astype(np.int32)  # [B, LS]
    tgt = np.asarray(inputs["target_source"]).astype(np.int32)  # [B, LT]
    emb = np.asarray(inputs["emb_W"], dtype=np.float32).copy()
    emb[0, :] = 0.0  # padding_idx
    emb_bf = np.zeros((V, EP), dtype=bf)
    emb_bf[:, :E] = emb.astype(bf)
    emb_bf[:, E] = 1.0  # multiplies the bias row of wih (k=2, row 44)

    def pack_wih(nm):
        W = np.asarray(inputs[f"Wih_{nm}"], dtype=np.float32).T.copy()  # [300, 1024]
        W[:, 2 * H : 3 * H] *= 2.0  # g-gate: tanh via sigmoid
        e4 = ml_dtypes.float8_e4m3
        pack = np.zeros((3, 128, G4), dtype=e4)
        pack[0] = W[0:128].astype(e4)
        pack[1] = W[128:256].astype(e4)
        pack[2, 64:108] = W[256:300].astype(e4)
        return pack

    def pack_bias(nm):
        bias = (
            np.asarray(inputs[f"bih_{nm}"], dtype=np.float32)
            + np.asarray(inputs[f"bhh_{nm}"], dtype=np.float32)
        ).copy()
        bias[2 * H : 3 * H] *= 2.0
        return np.ascontiguousarray(bias.reshape(8, 128).astype(bf))

    def pack_whh(nm):
        W = np.asarray(inputs[f"Whh_{nm}"], dtype=np.float32).T.copy()  # [256, 1024]
        W *= 2.0  # hidden states are stored halved
        W[:, 2 * H : 3 * H] *= 2.0
        return np.ascontiguousarray(W.reshape(2, 128, G4).astype(ml_dtypes.float8_e4m3))

    shared = {"emb": emb_bf}
    bx = np.zeros((8, 640), dtype=bf)
    for i, nm in enumerate(("sf", "sb", "tf", "tb")):
        shared[f"wih_{nm}"] = pack_wih(nm)
        shared[f"whh_{nm}"] = pack_whh(nm)
        bx[:, 128 * i : 128 * (i + 1)] = pack_bias(nm)
    bx[:, 512:640] = np.repeat(np.eye(8), BL, axis=1).astype(bf)
    shared["bx"] = np.ascontiguousarray(bx)
    Wout = np.asarray(inputs["Wout"], dtype=np.float32) * 2.0  # [3, 512]; sen_h halved
    shared["woutT"] = np.ascontiguousarray(Wout.T.reshape(4, 128, OUT).astype(bf))
    shared["boutT"] = np.asarray(inputs["bout"], dtype=np.float32).reshape(OUT, 1)

    in_maps = []
    for c in range(NCORES):
        sl = slice(c * BL, (c + 1) * BL)
        m = dict(shared)
        # sentence chunk j = timesteps [8j, 8j+8) across all 16 batch elems,
        # time-major within the chunk (matches the xT free layout t*16+b)
        sh = sen[sl]  # [16, 128]
        cols = [tgt[sl].T.reshape(-1)]
        cols += [sh[:, TS_CH * j : TS_CH * (j + 1)].T.reshape(-1) for j in range(NCH_S)]
        m["idx"] = np.ascontiguousarray(np.stack(cols, axis=1))  # [128, 17]
        in_maps.append(m)
    return in_maps


def kernel(**inputs) -> np.ndarray:
    nc = _get_program()
    in_maps = prepare_in_maps(inputs)
    r = run_bass_kernel_spmd(nc, in_maps, core_ids=list(range(NCORES)))
    return np.concatenate([r.results[c]["out"] for c in range(NCORES)], axis=0)


if __name__ == "__main__":
    print("building program...")
    nc = _get_program()
    print("build OK")
